# revision 11
# baseline (speedup 1.0000x reference)
"""Distributed Trainium2 Bass kernel for nn_B12xPagedAttention.

Tensor-parallel over heads across 8 NeuronCores for QKV + attention:
core c owns KV head c and its GQA group of 4 Q heads, plus the matching
QKV-weight row shard.  The output projection is token-parallel: ONE
AllToAll reshards the attention output from head-sharded [T, 512] to
token-sharded [T/8, 4096], and each core computes its 256 tokens' full
O-proj row block (Wo streamed from HBM), so the kernel output is a row
shard that the host simply concatenates.

Why AllToAll: any head-sharded-attn -> full-output scheme must either
gather attention (each core receives 14 MB) or reduce O-proj partials
(same 14 MB on the wire).  Resharding by tokens moves only what each
core actually needs: 7/8 x 2.1 MB ~= 1.8 MB per core -- 8x less
inter-core traffic, with the same matmul work.

Per-core structure:

  for b in 0..3:
    load cached K^T / V for batch b (K pre-transposed host-side)
    QKV projection for b's 4 token tiles (token-major), per-head RMSNorm +
      partial RoPE, PE-transpose Q/K to [D, tok], scatter new V
    GQA attention for b's 4 local q heads: scores^T = K^T x Q^T on PE, exp
      on ACT (scale folded; RMSNorm bounds scores so no max pass), P x V
      with an appended ones-column in V giving softmax denominators free;
      attn^T [D, tok] tiles are DMA'd into the AllToAll send buffer,
      split by destination core (token block)
  ONE AllToAll (bf16) -> this core's [4096 feat, 256 tok] attention block
  token-sharded O-proj: stream Wo^T column slices from HBM, accumulate
    out[256, 4096] in PSUM, store bf16 (host casts to f32)

Block-level causal masking: each (kv chunk, query tile) block is dead /
full / partial; dead blocks are skipped in scores, exp and P x V, and only
partial (diagonal) blocks multiply a host-baked 128x128 0/1 mask.

All matmuls bf16 with f32 PSUM accumulation.  Host-side prep (not on the
HW timeline): weight transposes/sharding, cos/sin gather by positions,
per-head cache gather via page_table, cached-K transpose, mask tiles.
"""

import os
import sys
from contextlib import ExitStack

import numpy as np

sys.path.insert(0, "/opt/trn_rl_repo")

import ml_dtypes  # noqa: E402

import concourse.bass as bass  # noqa: E402
from concourse import bacc  # noqa: E402
import concourse.tile as tile  # noqa: E402
from concourse import mybir  # noqa: E402
from concourse.bass_utils import run_bass_kernel_spmd  # noqa: E402
from concourse.masks import make_identity  # noqa: E402

BF16 = mybir.dt.bfloat16
F32 = mybir.dt.float32
NPBF16 = ml_dtypes.bfloat16

HQ, HKV, D, RD = 32, 8, 128, 64
EPS = 1e-6
B, QL, HID = 4, 512, 4096
T = B * QL
PS, MP = 16, 64
KV = PS * MP  # 1024 slots per sequence
NCORES = 8
G = HQ // HKV  # q heads per kv head = per core
QKV_N = G * D + 2 * D  # 768 per-core qkv features
TSH = T // NCORES  # 256 tokens per core after the AllToAll reshard
NKC = HID // 128  # 32 contraction chunks
NTOK = T // 128  # 16 token tiles
TPB = QL // 128  # 4 token tiles per batch
NKVC = KV // 128  # 8 kv chunks per sequence
OJ = 256  # O-proj output column-group width (16 groups)
NOJ = HID // OJ
SCALE = 1.0 / float(np.sqrt(D))

LAST_RESULT = None  # stash of BassKernelResults for test harness
LAST_IN_MAPS = None


def _block_kind(cs_b: int, kc: int, mq: int) -> str:
    """Mask status of (kv chunk kc) x (query tile mq) for cache len cs_b.
    kv slot k visible to query j iff k <= cs_b + j."""
    lo, hi = kc * 128, kc * 128 + 127
    jlo, jhi = mq * 128, mq * 128 + 127
    if lo > cs_b + jhi:
        return "dead"
    if hi <= cs_b + jlo:
        return "full"
    return "partial"


def _classify(cs):
    kinds = {(b, kc, mq): _block_kind(int(cs[b]), kc, mq)
             for b in range(B) for kc in range(NKVC) for mq in range(TPB)}
    live_kc = {b: [kc for kc in range(NKVC)
                   if any(kinds[(b, kc, mq)] != "dead" for mq in range(TPB))]
               for b in range(B)}
    return kinds, live_kc


def _mask_tiles(cs):
    """Host-baked [128,128] 0/1 bf16 masks for the partial blocks, deduped.
    Returns (packed [128, U*128] array, {(b,kc,mq): u})."""
    kinds, _ = _classify(cs)
    uniq = {}
    index = {}
    tiles = []
    ii = np.arange(128)[:, None]
    jj = np.arange(128)[None, :]
    for (b, kc, mq), kind in kinds.items():
        if kind != "partial":
            continue
        m = (kc * 128 + ii <= int(cs[b]) + mq * 128 + jj).astype(NPBF16)
        key = m.tobytes()
        if key not in uniq:
            uniq[key] = len(tiles)
            tiles.append(m)
        index[(b, kc, mq)] = uniq[key]
    if not tiles:
        tiles.append(np.zeros((128, 128), NPBF16))
    packed = np.ascontiguousarray(np.concatenate(tiles, axis=1))
    return packed, index


def _build_graph(cs: np.ndarray, n_mask_tiles: int, repeat: int = 1,
                 ablate: str = ""):
    """ablate: comma-joined flags for timing ablations (never used for the
    real kernel): no_cc (skip collective), no_oproj, no_attn, no_qkv."""
    ab = set(ablate.split(",")) if ablate else set()
    nc = bacc.Bacc(None)
    kinds, live_kc = _classify(cs)

    hT = nc.declare_dram_parameter("hiddenT", [HID, T], BF16, isOutput=False)
    wqkvT = nc.declare_dram_parameter("wqkvT", [HID, QKV_N], BF16, isOutput=False)
    # full Wo^T [in-feat, out], streamed per column slice during O-proj
    woT = nc.declare_dram_parameter("woT", [HID, HID], BF16, isOutput=False)
    kTc = nc.declare_dram_parameter("kTc", [B, 128, KV], BF16, isOutput=False)
    vcache = nc.declare_dram_parameter("vcache", [B, KV, D], BF16, isOutput=False)
    pcos = nc.declare_dram_parameter("pcos", [T, RD // 2], F32, isOutput=False)
    psin = nc.declare_dram_parameter("psin", [T, RD // 2], F32, isOutput=False)
    dmasks = nc.declare_dram_parameter(
        "dmasks", [128, n_mask_tiles * 128], BF16, isOutput=False)
    wq_b = nc.declare_dram_parameter("wq_b", [128, D], F32, isOutput=False)
    wk_b = nc.declare_dram_parameter("wk_b", [128, D], F32, isOutput=False)
    # this core's 256-token output row block (bf16; host casts to f32)
    res0 = nc.declare_dram_parameter("res0", [TSH, HID], BF16, isOutput=True)
    outsc = nc.dram_tensor("outsc", [TSH, HID], BF16)  # non-final reps
    # AllToAll buffers, double-buffered by repeat parity.  send block j =
    # my 4 heads' attn^T for tokens [j*256, (j+1)*256); recv block j =
    # core j's heads (canonical features [512j, 512j+512)) for MY tokens.
    sendb = [nc.dram_tensor(f"sendb{i}", [NCORES, G * D, TSH], BF16)
             for i in range(2)]
    recvb = [nc.dram_tensor(f"recvb{i}", [NCORES, G * D, TSH], BF16)
             for i in range(2)]

    with tile.TileContext(nc) as tc, ExitStack() as es:
        const = es.enter_context(tc.tile_pool(name="const", bufs=1))
        wpool = es.enter_context(tc.tile_pool(name="wpool", bufs=1))
        persist = es.enter_context(tc.tile_pool(name="persist", bufs=1))
        hstream = es.enter_context(tc.tile_pool(name="hstream", bufs=3))
        work = es.enter_context(tc.tile_pool(name="work", bufs=3))
        probsp = es.enter_context(tc.tile_pool(name="probsp", bufs=2))
        outp = es.enter_context(tc.tile_pool(name="outp", bufs=2))
        wostr = es.enter_context(tc.tile_pool(name="wostr", bufs=2))
        # PSUM budget (8 banks): accq x2 + acck/po x2 + sc/tp x2 + pv x2
        psum_b = es.enter_context(tc.tile_pool(name="psumb", bufs=2, space="PSUM"))

        ident = const.tile([128, 128], BF16, tag="ident")
        make_identity(nc, ident[:])
        zero1 = const.tile([128, 1], F32, tag="zero1")
        nc.gpsimd.memset(zero1[:], 0.0)
        eps1 = const.tile([128, 1], F32, tag="eps1")
        nc.gpsimd.memset(eps1[:], float(EPS))
        nc.const_aps.aps[(F32, 0.0)] = zero1[:]
        nc.const_aps.aps[(F32, float(EPS))] = eps1[:]
        wqb_sb = const.tile([128, D], F32, tag="wqb")
        nc.sync.dma_start(out=wqb_sb[:], in_=wq_b[:])
        wkb_sb = const.tile([128, D], F32, tag="wkb")
        nc.sync.dma_start(out=wkb_sb[:], in_=wk_b[:])
        dm_sb = const.tile([128, n_mask_tiles * 128], BF16, tag="dm")
        nc.gpsimd.dma_start(out=dm_sb[:], in_=dmasks[:])

        # resident QKV weights
        w_sb = wpool.tile([128, NKC, QKV_N], BF16, tag="wqkv")
        for j in range(0, NKC, 4):
            nc.sync.dma_start(
                out=w_sb[:, j : j + 4, :],
                in_=wqkvT[j * 128 : (j + 4) * 128, :].rearrange(
                    "(kc p) n -> p kc n", p=128
                ),
            )

        # persistent attention operands (double-buffered by batch parity)
        kT = {b: persist.tile([128, KV], BF16, tag=f"kT{b}", name=f"kT{b}")
              for b in range(B)}
        vsb = {b: persist.tile([128, NKVC, D + 1], BF16, tag=f"v{b}", name=f"v{b}")
               for b in range(B)}
        qT = {(bp, h): persist.tile([128, QL], BF16, tag=f"qT{bp}_{h}",
                                    name=f"qT{bp}_{h}")
              for bp in range(2) for h in range(G)}
        # gathered attn^T [feat, tok] for this core's 256 tokens
        a_me = persist.tile([128, NKC, TSH], BF16, tag="ame", name="ame")

        def load_cached(b):
            cs_b = int(cs[b])
            if cs_b + QL < KV or (cs_b + QL) % 128:
                # tail kv slots beyond cs+QL are never visible but do enter
                # scores/PV of boundary chunks: zero them so exp stays finite
                nc.gpsimd.memset(kT[b][:, cs_b + QL : KV], 0.0)
                nc.gpsimd.memset(vsb[b][:, :, 0:D], 0.0)
            if cs_b > 0:
                nc.gpsimd.dma_start(out=kT[b][:, 0:cs_b], in_=kTc[b, :, 0:cs_b])
            nc.gpsimd.memset(vsb[b][:, :, D : D + 1], 1.0)
            ncache = (cs_b + 127) // 128  # cached v chunks (+boundary chunk)
            if ncache > 0:
                nc.gpsimd.dma_start(
                    out=vsb[b][:, 0:ncache, 0:D],
                    in_=vcache[b, 0 : ncache * 128, :].rearrange(
                        "(kc p) d -> p kc d", p=128
                    ),
                )

        def phase1_tile(b, mq):
            """QKV matmuls + norm/rope DVE-ACT chain for token tile mq of
            batch b.  Returns a finish() closure holding the PE transposes +
            copies."""
            ti = b * TPB + mq
            tloc = mq * 128
            cs_b = int(cs[b])
            bp = b % 2

            h_sb = hstream.tile([128, NKC, 128], BF16, tag="h")
            h_dma = nc.scalar.dma_start if mq % 2 else nc.sync.dma_start
            h_dma(
                out=h_sb[:],
                in_=hT[:, ti * 128 : (ti + 1) * 128].rearrange(
                    "(kc p) t -> p kc t", p=128
                ),
            )
            accq = psum_b.tile([128, G * D], F32, tag="accq", name="accq")
            acck = psum_b.tile([128, 2 * D], F32, tag="acck", name="acck",
                               bufs=2)
            for kc in range(NKC):
                nc.tensor.matmul(
                    accq[:], lhsT=h_sb[:, kc, :],
                    rhs=w_sb[:, kc, 0 : G * D],
                    start=(kc == 0), stop=(kc == NKC - 1),
                )
                nc.tensor.matmul(
                    acck[:], lhsT=h_sb[:, kc, :],
                    rhs=w_sb[:, kc, G * D : QKV_N],
                    start=(kc == 0), stop=(kc == NKC - 1),
                )

            pc_sb = work.tile([128, RD // 2], F32, tag="pc")
            nc.gpsimd.dma_start(out=pc_sb[:], in_=pcos[ti * 128 : (ti + 1) * 128, :])
            ps_sb = work.tile([128, RD // 2], F32, tag="ps")
            nc.gpsimd.dma_start(out=ps_sb[:], in_=psin[ti * 128 : (ti + 1) * 128, :])

            def norm_rope(src_ap, nh, w_bcast):
                """src_ap: [128 tok, nh, D] psum view; per-head RMSNorm + RoPE
                batched over nh heads; returns the rotated SBUF tile."""
                RH = RD // 2
                sq = work.tile([128, nh, D], F32, tag=f"sq{nh}", name="sq")
                nc.scalar.activation(
                    out=sq[:], in_=src_ap, func=mybir.ActivationFunctionType.Square
                )
                ssum = work.tile([128, nh, 1], F32, tag=f"ssum{nh}", name="ssum")
                nc.vector.reduce_sum(out=ssum[:], in_=sq[:], axis=mybir.AxisListType.X)
                rstd = work.tile([128, nh, 1], F32, tag=f"rstd{nh}", name="rstd")
                nc.scalar.activation(
                    out=rstd[:], in_=ssum[:],
                    func=mybir.ActivationFunctionType.Sqrt,
                    scale=1.0 / D, bias=float(EPS),
                )
                nc.vector.reciprocal(out=rstd[:], in_=rstd[:])
                qn = work.tile([128, nh, D], F32, tag=f"qn{nh}", name="qn")
                nc.vector.tensor_mul(
                    out=qn[:], in0=src_ap, in1=rstd[:].to_broadcast([128, nh, D])
                )
                nc.vector.tensor_mul(
                    out=qn[:], in0=qn[:],
                    in1=w_bcast[:].unsqueeze(1).to_broadcast([128, nh, D]),
                )
                ro = work.tile([128, nh, D], BF16, tag=f"ro{nh}", name="ro")
                cb = pc_sb[:].unsqueeze(1).to_broadcast([128, nh, RH])
                sb = ps_sb[:].unsqueeze(1).to_broadcast([128, nh, RH])
                t1 = work.tile([128, nh, RH], F32, tag=f"t1{nh}", name="t1")
                t2 = work.tile([128, nh, RH], F32, tag=f"t2{nh}", name="t2")
                nc.vector.tensor_mul(out=t1[:], in0=qn[:, :, 0:RH], in1=cb)
                nc.vector.tensor_mul(out=t2[:], in0=qn[:, :, RH:RD], in1=sb)
                nc.vector.tensor_sub(out=ro[:, :, 0:RH], in0=t1[:], in1=t2[:])
                nc.vector.tensor_mul(out=t1[:], in0=qn[:, :, RH:RD], in1=cb)
                nc.vector.tensor_mul(out=t2[:], in0=qn[:, :, 0:RH], in1=sb)
                nc.vector.tensor_add(out=ro[:, :, RH:RD], in0=t1[:], in1=t2[:])
                nc.scalar.activation(
                    out=ro[:, :, RD:D], in_=qn[:, :, RD:D],
                    func=mybir.ActivationFunctionType.Copy,
                )
                return ro

            qv = accq[:].rearrange("p (h d) -> p h d", h=G)
            ro_q = norm_rope(qv, G, wqb_sb)
            kv_view = acck[:, 0:D].rearrange("p (h d) -> p h d", h=1)
            ro_k = norm_rope(kv_view, 1, wkb_sb)
            # new V -> bf16 staging tile (ACT), scattered in finish()
            vnew = work.tile([128, D], BF16, tag="vnew")
            nc.scalar.activation(
                out=vnew[:], in_=acck[:, D : 2 * D],
                func=mybir.ActivationFunctionType.Copy,
            )

            def finish():
                dsts = [qT[(bp, h)][:, tloc : tloc + 128] for h in range(G)]
                dsts.append(kT[b][:, cs_b + tloc : cs_b + tloc + 128])
                srcs = [ro_q[:, h, :] for h in range(G)] + [ro_k[:, 0, :]]
                for src, dst in zip(srcs, dsts):
                    tp = psum_b.tile([128, 128], BF16, tag="sc", name="tp",
                                     bufs=2)
                    nc.tensor.transpose(tp[:], src, ident[:])
                    nc.scalar.activation(
                        out=dst, in_=tp[:],
                        func=mybir.ActivationFunctionType.Copy,
                    )
                r0 = cs_b + tloc  # global kv row of vnew partition 0
                off = r0 % 128
                c0 = r0 // 128
                if off == 0:
                    nc.sync.dma_start(out=vsb[b][0:128, c0, 0:D], in_=vnew[:])
                else:
                    n1 = 128 - off
                    nc.sync.dma_start(
                        out=vsb[b][off : off + n1, c0, 0:D], in_=vnew[0:n1, :]
                    )
                    nc.sync.dma_start(
                        out=vsb[b][0:off, c0 + 1, 0:D], in_=vnew[n1:128, :]
                    )

            return finish

        def attn_scores(b, h, mask_index):
            """scores + exp + diagonal masks for one head; returns probs."""
            bp = b % 2
            probs = {}
            for kc in live_kc[b]:
                mq_live = [mq for mq in range(TPB)
                           if kinds[(b, kc, mq)] != "dead"]
                q0 = mq_live[0] * 128
                sc = psum_b.tile([128, QL], F32, tag="sc", name="sc", bufs=2)
                nc.tensor.matmul(
                    sc[:, q0:QL], lhsT=kT[b][:, kc * 128 : (kc + 1) * 128],
                    rhs=qT[(bp, h)][:, q0:QL], start=True, stop=True,
                )
                pr = probsp.tile([128, QL], BF16, tag=f"pr{kc}", bufs=2)
                probs[kc] = pr
                nc.scalar.activation(
                    out=pr[:, q0:QL], in_=sc[:, q0:QL],
                    func=mybir.ActivationFunctionType.Exp, scale=SCALE,
                )
                for mq in mq_live:
                    if kinds[(b, kc, mq)] == "partial":
                        u = mask_index[(b, kc, mq)]
                        nc.vector.tensor_mul(
                            out=pr[:, mq * 128 : (mq + 1) * 128],
                            in0=pr[:, mq * 128 : (mq + 1) * 128],
                            in1=dm_sb[:, u * 128 : (u + 1) * 128],
                        )
            return probs

        def attn_pv(b, h, probs, stgt):
            """P x V for one head; attn^T [D, QL] goes to the AllToAll send
            buffer, split by destination core (256-token block)."""
            att = outp.tile([128, QL], BF16, tag="att")

            def pv_finish(mq, pv):
                rec = work.tile([128, 1], F32, tag="rec")
                nc.vector.reciprocal(out=rec[:], in_=pv[:, D : D + 1])
                sat = work.tile([128, D], BF16, tag="sat")
                nc.vector.tensor_scalar_mul(
                    out=sat[:], in0=pv[:, 0:D], scalar1=rec[:]
                )
                tp = psum_b.tile([128, 128], BF16, tag="pv", name="tp")
                nc.tensor.transpose(tp[:], sat[:], ident[:])
                nc.vector.tensor_copy(att[:, mq * 128 : (mq + 1) * 128], tp[:])

            for mq in range(TPB):
                kcs = [kc for kc in live_kc[b] if kinds[(b, kc, mq)] != "dead"]
                pv = psum_b.tile([128, D + 1], F32, tag="pv")
                for i, kc in enumerate(kcs):
                    nc.tensor.matmul(
                        pv[:], lhsT=probs[kc][:, mq * 128 : (mq + 1) * 128],
                        rhs=vsb[b][:, kc, :],
                        start=(i == 0), stop=(i == len(kcs) - 1),
                    )
                pv_finish(mq, pv)
            # tokens [512b, 512b+256) -> core 2b, next 256 -> core 2b+1
            nc.sync.dma_start(
                out=stgt[2 * b, h * D : (h + 1) * D, :], in_=att[:, 0:TSH]
            )
            nc.scalar.dma_start(
                out=stgt[2 * b + 1, h * D : (h + 1) * D, :], in_=att[:, TSH:QL]
            )

        def attention(b, h, mask_index, stgt):
            attn_pv(b, h, attn_scores(b, h, mask_index), stgt)

        def oproj(rtgt, rcv):
            """Token-sharded O-proj: out[256, HID] = attn_me.T @ Wo^T with
            Wo^T column slices streamed from HBM."""
            nc.gpsimd.dma_start(
                out=a_me[:],
                in_=rcv[:, :, :].rearrange("j (k p) t -> p (j k) t", p=128),
            )
            o_sb = {tt: outp.tile([128, HID], BF16, tag=f"osb{tt}", bufs=1,
                                  name=f"osb{tt}")
                    for tt in range(2)}
            for j in range(NOJ):
                woj = wostr.tile([128, NKC, OJ], BF16, tag="woj")
                nc.scalar.dma_start(
                    out=woj[:],
                    in_=woT[:, j * OJ : (j + 1) * OJ].rearrange(
                        "(kc p) n -> p kc n", p=128
                    ),
                )
                po = {tt: psum_b.tile([128, OJ], F32, tag="acck", name="po",
                                      bufs=2)
                      for tt in range(2)}
                for kc in range(NKC):
                    for tt in range(2):
                        nc.tensor.matmul(
                            po[tt][:],
                            lhsT=a_me[:, kc, tt * 128 : (tt + 1) * 128],
                            rhs=woj[:, kc, :],
                            start=(kc == 0), stop=(kc == NKC - 1),
                        )
                for tt in range(2):
                    if j % 2:
                        nc.vector.tensor_copy(
                            o_sb[tt][:, j * OJ : (j + 1) * OJ], po[tt][:]
                        )
                    else:
                        nc.scalar.activation(
                            out=o_sb[tt][:, j * OJ : (j + 1) * OJ],
                            in_=po[tt][:],
                            func=mybir.ActivationFunctionType.Copy,
                        )
            for tt in range(2):
                hh = HID // 2
                nc.sync.dma_start(
                    out=rtgt[tt * 128 : (tt + 1) * 128, 0:hh],
                    in_=o_sb[tt][:, 0:hh],
                )
                nc.gpsimd.dma_start(
                    out=rtgt[tt * 128 : (tt + 1) * 128, hh:HID],
                    in_=o_sb[tt][:, hh:HID],
                )

        mask_index = _mask_tiles(cs)[1]

        for _rep in range(repeat):  # repeat>1 only for steady-state benching
            stgt = sendb[_rep % 2]
            rcv = recvb[_rep % 2]
            rtgt = res0 if _rep == repeat - 1 else outsc
            for b in range(B):
                load_cached(b)
            for b in range(B):
                if "no_qkv" not in ab:
                    for mq in range(TPB):
                        phase1_tile(b, mq)()
                if "no_attn" not in ab:
                    for h in range(G):
                        attention(b, h, mask_index, stgt)
            if "no_cc" not in ab:
                nc.gpsimd.collective_compute(
                    "AllToAll",
                    mybir.AluOpType.bypass,
                    ins=[stgt[:, :, :]],
                    outs=[rcv[:, :, :]],
                    replica_groups=[list(range(NCORES))],
                )
            if "no_oproj" not in ab:
                oproj(rtgt, rcv)

    nc.finalize()
    return nc


def kernel(
    hidden_states, cos, sin, positions, k_cache, v_cache, page_table,
    cache_seqlens, cu_seqlens_q, qkv_weight, o_proj_weight,
    q_norm_weight, k_norm_weight,
):
    global LAST_RESULT, LAST_IN_MAPS
    hidden_states = np.asarray(hidden_states)
    cs = np.asarray(cache_seqlens).astype(np.int64)
    positions = np.asarray(positions).astype(np.int64)
    page_table = np.asarray(page_table).astype(np.int64)
    k_cache = np.asarray(k_cache)
    v_cache = np.asarray(v_cache)
    qkv_weight = np.asarray(qkv_weight)
    o_proj_weight = np.asarray(o_proj_weight)
    cos = np.asarray(cos)
    sin = np.asarray(sin)

    hiddenT = np.ascontiguousarray(hidden_states.T).astype(NPBF16)
    pc = np.ascontiguousarray(cos[positions]).astype(np.float32)
    psn = np.ascontiguousarray(sin[positions]).astype(np.float32)
    wq_b = np.ascontiguousarray(
        np.broadcast_to(np.asarray(q_norm_weight, np.float32)[None, :], (128, D))
    )
    wk_b = np.ascontiguousarray(
        np.broadcast_to(np.asarray(k_norm_weight, np.float32)[None, :], (128, D))
    )
    woT_full = np.ascontiguousarray(o_proj_weight.T).astype(NPBF16)  # [HID, HID]

    packed_masks, _ = _mask_tiles(cs)
    n_mask_tiles = packed_masks.shape[1] // 128

    # per-sequence effective cache gather via page_table (per kv head below)
    flat_pages = page_table.reshape(-1)  # [B*MP]
    kc_seq = k_cache[flat_pages].reshape(B, KV, HKV, D)
    vc_seq = v_cache[flat_pages].reshape(B, KV, HKV, D)

    in_maps = []
    for c in range(NCORES):
        qrows = qkv_weight[c * G * D : (c + 1) * G * D]  # [512, HID]
        krow = qkv_weight[HQ * D + c * D : HQ * D + (c + 1) * D]
        vrow = qkv_weight[(HQ + HKV) * D + c * D : (HQ + HKV) * D + (c + 1) * D]
        wT = np.ascontiguousarray(
            np.concatenate([qrows, krow, vrow], axis=0).T
        ).astype(NPBF16)  # [HID, 768]
        kTc = np.ascontiguousarray(
            kc_seq[:, :, c, :].transpose(0, 2, 1)
        ).astype(NPBF16)  # [B, D, KV]
        in_maps.append(
            dict(
                hiddenT=hiddenT,
                wqkvT=wT,
                woT=woT_full,
                kTc=kTc,
                vcache=np.ascontiguousarray(vc_seq[:, :, c, :]).astype(NPBF16),
                pcos=pc,
                psin=psn,
                dmasks=packed_masks,
                wq_b=wq_b,
                wk_b=wk_b,
            )
        )

    LAST_IN_MAPS = in_maps
    nc = _build_graph(cs, n_mask_tiles)
    res = run_bass_kernel_spmd(
        nc, in_maps, core_ids=list(range(NCORES)),
        trace=bool(os.environ.get("BASS_TRACE")),
    )
    LAST_RESULT = res
    return np.concatenate(
        [np.asarray(r["res0"]).astype(np.float32) for r in res.results], axis=0
    )
